# revision 57
# baseline (speedup 1.0000x reference)
"""HEALPix downsample (scatter-mean over parent_map + 1x1 conv) on 8 Trainium2 cores.

Strategy (data-parallel over the 16 (B*C*T) slices, 2 per core):
  - Host: sort source rows by parent, pack x fp8-e3m4 (x2.5 global scale,
    max |val| ~13.5 < 15.5) with the 1/count mean weights folded into the
    rows (no padding: 49152 = 384*128). The device scatter is then a pure
    one-hot sum; fp8 halves the dominant x HBM read vs fp16 (rel err
    ~1.2e-2 vs the 2e-2 gate; e3m4's 4 mantissa bits are required - e4m3
    fails at 2.6e-2). The 1/scale is folded into the fp16 conv weight.
  - A-matrices: per 128-subtile chunk, ONE DVE is_equal op with broadcast
    APs builds all one-hot blocks [128, 128*BW] (iota tiled vs band-relative
    targets). x-independent, so off the DMA critical path.
  - Scatter: per subtile one narrow fp16 matmul psum[d, band] += X_t^T @ A_t
    into a 512-target PSUM window (one bank); bands crossing a window
    boundary issue two column-sliced matmuls. Windows zeroed by ScalarEngine.
  - 1x1 conv: fp16 matmul with W^T stationary; bias fused into the ACT
    PSUM->SBUF copy which writes fp16 output (halved output DMA).
  - The window loop is software-pipelined (tail ops lag LAG=2 windows behind
    the scatter matmuls) so no engine stalls on another's just-issued work;
    on real HW the naive per-window chain serializes and costs ~4x.
  - x chunk DMAs alternate between the SP and ACT HWDGE rings; few large
    DMAs (6 x-chunks + 6 outputs per repeat) beat many small ones.
Output is fp16 in DRAM; host upcasts to f32.
"""

import numpy as np
from contextlib import ExitStack

import concourse.bacc as bacc
import concourse.tile as tile
from concourse import mybir
from concourse.bass_utils import run_bass_kernel_spmd

# ---- problem constants (hardcoded; kernel.py must be self-contained) ----
B, C, T = 2, 2, 4
S = B * C * T                 # 16 flattened batch slices
N_SRC = 49152
N_TGT = 12288
D = 128
N_CORES = 8
S_PER_CORE = S // N_CORES     # 2

P = 128                       # partitions
NSUB = N_SRC // P             # 384 subtiles per slice (no padding)
G = 512                       # targets per psum window (one full PSUM bank)
NWIN = N_TGT // G             # 24 windows per slice
CHUNK = 128
X_PIECES = 4                  # DMA pieces per x chunk (2 -> 1MB pieces)
OUT_WINS = 4

F32 = mybir.dt.float32
F16 = mybir.dt.float16
F8 = mybir.dt.float8e3          # e3m4: 4 mantissa bits, max 15.5
SCALE = 2.5                     # folded into x on host, undone via conv weight

# experiment knobs
# GPSIMD cannot touch PSUM (HW restriction), so the three per-window PSUM
# passes (memzero, mean copy, bias+convert) are split evenly across ACT and
# DVE by window parity; Pool builds the A one-hots (SBUF-only work).
MZ_PATTERN = ("scalar", "vector")
CP_PATTERN = ("vector", "scalar")
BIAS_PATTERN = ("scalar", "vector")
# A chunks are built on DVE (Pool rejects TensorTensor) in half-chunk
# pieces, interleaved into the first windows of slice 0; while they run,
# the copy/bias tail stays off DVE entirely.
AB_ENGINE = "vector"
AB_AT = {0: [(0, 0), (0, 1)], 2: [(1, 0)], 5: [(1, 1)], 8: [(2, 0)],
         11: [(2, 1)]}
AB_WINDOWS = 16
# Ring split by direction: sync ring streams x-chunk loads back-to-back
# (issued up front per slice; all 6 chunk tiles are SBUF-resident), scalar
# ring takes consts + output stores.  Sharing a FIFO ring between loads and
# compute-gated stores head-of-line blocks the loads.
OUT_QUEUE = "sync"            # "sync" | "scalar"
LAG = 2                       # windows between scatter matmuls and copy/conv
BLAG = 1                      # extra windows between conv and bias/out


def _plan(parent_map):
    """Host-side metadata: packed row order, band-relative targets, matmuls."""
    pm = np.asarray(parent_map).astype(np.int64).ravel()
    assert pm.shape == (N_SRC,)
    perm = np.argsort(pm, kind="stable")
    sp = pm[perm]
    cnt = np.bincount(pm, minlength=N_TGT)
    inv = np.where(cnt > 0, 1.0 / np.maximum(cnt, 1), 0.0).astype(np.float32)

    l = sp.reshape(NSUB, P)                      # targets per (subtile, row)
    tmin = l.min(axis=1)
    tmax = l.max(axis=1)
    span = tmax - tmin + 1
    BW = int(max(32, -(-int(span.max()) // 16) * 16))   # uniform A-block width
    assert BW <= G, f"pathological parent_map: span {span.max()} > {G}"

    rel = (l - tmin[:, None]).T.astype(np.float32)      # [128, NSUB], in [0, BW)
    wrow = inv[sp]                                      # folded mean weights

    # per-subtile matmul schedule: (t, a_lo, a_hi, w, psum_off)
    sched = []
    for t in range(NSUB):
        lo = int(tmin[t])
        bw_t = min(BW, -(-int(span[t]) // 16) * 16)
        w = lo // G
        wend = (w + 1) * G
        if int(tmax[t]) < wend:
            bw_t = min(bw_t, wend - lo)               # clip phantom overhang
            sched.append((t, 0, bw_t, w, lo - w * G))
        else:                                          # band crosses window end
            cut = wend - lo
            sched.append((t, 0, cut, w, lo - w * G))
            sched.append((t, cut, bw_t, w + 1, 0))
    by_win_raw = [[] for _ in range(NWIN)]
    for m in sched:
        by_win_raw[m[3]].append(m)

    # Start-flag coverage: rewrite each window's bands into pieces such that
    # every psum column is written exactly once with start=True (the first
    # band covering it; gaps over source-less targets are covered by the
    # previous band's phantom all-zero A columns) and accumulated into by
    # the rest.  Removes the per-window PSUM memzero pass entirely.
    by_win = []
    needs_mz = []
    for w in range(NWIN):
        pieces = []
        cover = w * G
        ok = True
        prev = None                                  # (t, tmin[t]) last band
        if w > 0 and by_win_raw[w - 1]:
            tl = by_win_raw[w - 1][-1][0]
            prev = (tl, int(tmin[tl]))
        for (t, a_lo, a_hi, _, poff) in by_win_raw[w]:
            tb = int(tmin[t])
            lo_g, hi_g = tb + a_lo, tb + a_hi
            if lo_g > cover:                         # gap: zero-fill piece
                if prev is None or lo_g - prev[1] > BW:
                    ok = False
                    break
                pieces.append((prev[0], cover - prev[1], lo_g - prev[1],
                               cover - w * G, True))
            ov = max(0, min(cover, hi_g) - lo_g)
            if ov > 0:
                pieces.append((t, a_lo, a_lo + ov, lo_g - w * G, False))
            if hi_g > cover:
                st = max(cover, lo_g)
                pieces.append((t, a_lo + (st - lo_g), a_hi, st - w * G, True))
                cover = hi_g
            prev = (t, tb)
        if ok and cover < (w + 1) * G:               # tail gap
            end_g = (w + 1) * G
            if prev is None or end_g - prev[1] > BW:
                ok = False
            else:
                pieces.append((prev[0], cover - prev[1], end_g - prev[1],
                               cover - w * G, True))
        if not ok:                                   # rare fallback: memzero
            pieces = [(t, a_lo, a_hi, poff, False)
                      for (t, a_lo, a_hi, _, poff) in by_win_raw[w]]
        needs_mz.append(not ok)
        by_win.append(pieces)
    return perm, wrow, rel, (by_win, needs_mz), BW


def _build(rel_np, by_win_mz, BW, x_bufs=4, repeats=1):
    by_win, needs_mz = by_win_mz
    nc = bacc.Bacc("TRN2", target_bir_lowering=False, debug=False, enable_asserts=False)
    FREE = NSUB * P
    xp = nc.dram_tensor("xp", [S_PER_CORE, P, FREE], F8, kind="ExternalInput").ap()
    # one packed f16 const block: [rel | iota | wt | bias(f32 as 2xf16)]
    CW = NSUB + BW + P + 2
    cst = nc.dram_tensor("cst", [P, CW], F16, kind="ExternalInput").ap()
    out = nc.dram_tensor("out", [S_PER_CORE, P, N_TGT], F16, kind="ExternalOutput").ap()

    eq = mybir.AluOpType.is_equal
    n_chunks = -(-NSUB // CHUNK)

    def mz(eng_name, ap):
        if eng_name == "scalar":
            nc.scalar.memzero(ap)
        else:
            nc.vector.memset(ap, 0.0)

    out_eng = {"sync": nc.sync, "scalar": nc.scalar}[OUT_QUEUE]

    def cp(eng_name, out_ap, in_ap):
        if eng_name == "scalar":
            nc.scalar.copy(out=out_ap, in_=in_ap)
        else:
            eng = {"vector": nc.vector, "gpsimd": nc.gpsimd}[eng_name]
            eng.tensor_copy(out=out_ap, in_=in_ap)

    def bias_add(eng_name, out_ap, in_ap, bias_ap):
        if eng_name == "scalar":
            nc.scalar.add(out_ap, in_ap, bias_ap)
        else:
            eng = {"vector": nc.vector, "gpsimd": nc.gpsimd}[eng_name]
            eng.tensor_scalar(out=out_ap, in0=in_ap, scalar1=bias_ap,
                              scalar2=None, op0=mybir.AluOpType.add)

    with ExitStack() as ctx:
        tc = ctx.enter_context(tile.TileContext(nc))
        cpool = ctx.enter_context(tc.tile_pool(name="const", bufs=1))
        xpool = ctx.enter_context(tc.tile_pool(name="x", bufs=1))
        mpool = ctx.enter_context(tc.tile_pool(name="mean", bufs=3))
        opool = ctx.enter_context(tc.tile_pool(name="osb", bufs=1))
        pp = ctx.enter_context(tc.tile_pool(name="pp", bufs=LAG + 2, space="PSUM"))
        po = ctx.enter_context(tc.tile_pool(name="po", bufs=3, space="PSUM"))

        cst_t = cpool.tile([P, CW], F16, tag="cst")
        nc.sync.dma_start(out=cst_t[:], in_=cst[:])
        rl_t = cst_t[:, 0:NSUB]
        io_t = cst_t[:, NSUB:NSUB + BW]
        wt_t = cst_t[:, NSUB + BW:NSUB + BW + P]
        bi_t = cst_t[:, NSUB + BW + P:NSUB + BW + P + 2].bitcast(F32)

        # A is slice-invariant (parent_map only): one resident SBUF tile per
        # chunk, filled once (half-chunk builds scheduled via AB_AT below)
        # and reused for all slices/repeats.
        a_tiles = [
            cpool.tile([P, (min(FREE, (c + 1) * CHUNK * P) // P - c * CHUNK) * BW],
                       F8, tag=f"at{c}", name=f"at{c}")
            for c in range(n_chunks)
        ]

        ab_eng = {"vector": nc.vector, "gpsimd": nc.gpsimd}[AB_ENGINE]

        def build_a(c, half):
            nsub_c = a_tiles[c].shape[1] // BW
            h0 = half * (nsub_c // 2)
            h1 = nsub_c if half else nsub_c // 2
            nh = h1 - h0
            ab_eng.tensor_tensor(
                out=a_tiles[c][:, h0 * BW:h1 * BW]
                    .rearrange("p (t c) -> p t c", t=nh),
                in0=io_t[:].unsqueeze(1).broadcast_to([P, nh, BW]),
                in1=rl_t[:, c * CHUNK + h0:c * CHUNK + h1]
                    .unsqueeze(2).broadcast_to([P, nh, BW]),
                op=eq)

        for rep in range(repeats):
            # issue the whole repeat's x stream (both slices) up front on the
            # sync ring so loads never queue behind compute-gated output
            # stores; every piece has its own resident buffer.  The first
            # chunk arrives as two 1MB halves (fast pipeline fill), the rest
            # as full 2MB chunks (better HBM efficiency).
            x_tiles = {}
            xspan = CHUNK // X_PIECES
            for s in range(S_PER_CORE):
                for c in range(n_chunks):
                    parts = []
                    for h in range(X_PIECES):
                        f0 = (c * CHUNK + h * xspan) * P
                        f1 = min(FREE, f0 + xspan * P)
                        xt = xpool.tile([P, f1 - f0], F8, tag=f"xc{s}_{c}_{h}",
                                        name=f"xc{s}_{c}_{h}")
                        nc.sync.dma_start(out=xt[:], in_=xp[s, :, f0:f1])
                        parts.append(xt)
                    x_tiles[(s, c)] = parts

            def x_slice(s, c, loc):
                h, lh = divmod(loc, xspan)
                return x_tiles[(s, c)][h][:, lh * P:(lh + 1) * P]

            # software-pipelined, two-stage tail: iter w issues the scatter
            # matmuls for window w, the copy+conv for window w-LAG, and the
            # bias+out-DMA for window w-LAG-BLAG.  Splitting copy and bias
            # into different lag stages keeps ACT from stalling on the PE
            # conv it just fed (the conv sits behind window w's scatter
            # matmuls in PE program order).
            for s in range(S_PER_CORE):
                ps_tiles = {}
                pso_tiles = {}
                # full-slice output staging: biases land here unconditionally
                # (never blocked on out-DMA completion); the sync ring drains
                # OUT_WINS-window pieces behind the x stream at its own pace.
                out_t = opool.tile([P, N_TGT], F16, tag=f"os{s}", name=f"os{s}")
                for w in range(NWIN + LAG + BLAG):
                    first = rep == 0 and s == 0
                    if first:
                        for c_h in AB_AT.get(w, ()):
                            build_a(*c_h)
                    if w < NWIN:
                        mlist = by_win[w]
                        ps = pp.tile([P, G], F32, tag="pp")
                        if needs_mz[w]:
                            mz(MZ_PATTERN[w % len(MZ_PATTERN)], ps[:])
                        ps_tiles[w] = ps
                        nlast = len(mlist) - 1
                        for i, (t, a_lo, a_hi, poff, st) in enumerate(mlist):
                            c, loc = divmod(t, CHUNK)
                            nc.tensor.matmul(
                                out=ps[:, poff:poff + (a_hi - a_lo)],
                                lhsT=x_slice(s, c, loc),
                                rhs=a_tiles[c][:, loc * BW + a_lo:loc * BW + a_hi],
                                start=st, stop=(i == nlast),
                                skip_group_check=True)
                    wt_ = w - LAG
                    if 0 <= wt_ < NWIN:
                        ps_ = ps_tiles.pop(wt_)
                        mean_t = mpool.tile([P, G], F16, tag="mt")
                        cpe = ("scalar" if first and wt_ < AB_WINDOWS
                               else CP_PATTERN[wt_ % len(CP_PATTERN)])
                        cp(cpe, mean_t[:], ps_[:])
                        pso = po.tile([P, G], F32, tag="po")
                        nc.tensor.matmul(out=pso[:], lhsT=wt_t, rhs=mean_t[:],
                                         start=True, stop=True)
                        pso_tiles[wt_] = pso
                    wb = w - LAG - BLAG
                    if wb < 0:
                        continue
                    be = ("scalar" if first and wb < AB_WINDOWS
                          else BIAS_PATTERN[wb % len(BIAS_PATTERN)])
                    bias_add(be, out_t[:, wb * G:(wb + 1) * G],
                             pso_tiles.pop(wb)[:], bi_t[:, 0:1])
                    if wb % OUT_WINS == OUT_WINS - 1:
                        w0 = wb - (OUT_WINS - 1)
                        out_eng.dma_start(out=out[s, :, w0 * G:(wb + 1) * G],
                                          in_=out_t[:, w0 * G:(wb + 1) * G])
    nc.compile()
    return nc


_CACHE = {}


def _prepare(parent_map):
    key = np.asarray(parent_map).astype(np.int64).tobytes()
    entry = _CACHE.get(key)
    if entry is None:
        perm, wrow, rel, by_win, BW = _plan(parent_map)
        nc = _build(rel, by_win, BW)
        entry = (nc, perm, wrow, rel, BW)
        _CACHE[key] = entry
        _CACHE[(key, "plan")] = (perm, wrow, rel, by_win, BW)
    return entry


def build_repeated(parent_map, repeats):
    """Benchmark variant: same program with the whole body repeated."""
    _prepare(parent_map)
    key = np.asarray(parent_map).astype(np.int64).tobytes()
    perm, wrow, rel, by_win, BW = _CACHE[(key, "plan")]
    return _build(rel, by_win, BW, repeats=repeats)


def make_in_maps(x, parent_map, W, b):
    """Pack full inputs into the 8 per-core input maps."""
    import ml_dtypes
    f8 = ml_dtypes.float8_e3m4
    nc, perm, wrow, rel, BW = _prepare(parent_map)
    x2 = np.asarray(x, np.float32).reshape(S, N_SRC, D)
    FREE = NSUB * P
    wcol = (wrow.astype(np.float32) * SCALE)[:, None]
    xp_all = np.empty((S, P, FREE), f8)
    for s in range(S):
        g = (x2[s][perm] * wcol).astype(f8).reshape(NSUB, P, D)
        xp_all[s] = g.transpose(1, 0, 2).reshape(P, FREE)
    wt = (np.asarray(W, np.float32).T / SCALE).astype(np.float16)
    bias = np.asarray(b, np.float32).reshape(P, 1)
    iota = np.broadcast_to(np.arange(BW, dtype=np.float16), (P, BW))
    rel_c = rel.astype(np.float16)
    cst = np.ascontiguousarray(np.concatenate(
        [rel_c, iota, wt, bias.view(np.float16)], axis=1))
    in_maps = []
    for c in range(N_CORES):
        in_maps.append({
            "xp": np.ascontiguousarray(xp_all[c * S_PER_CORE:(c + 1) * S_PER_CORE]),
            "cst": cst,
        })
    return nc, in_maps


def assemble_output(results):
    """results: per-core list of {"out": [S_PER_CORE, P, N_TGT]} -> full output."""
    outs = np.stack([np.asarray(results[c]["out"]) for c in range(N_CORES)])
    out_full = outs.astype(np.float32).reshape(S, P, N_TGT).transpose(0, 2, 1)
    return np.ascontiguousarray(out_full).reshape(B, C, T, N_TGT, D)


def kernel(x, parent_map, W, b):
    nc, in_maps = make_in_maps(x, parent_map, W, b)
    res = run_bass_kernel_spmd(nc, in_maps, list(range(N_CORES)))
    return assemble_output(res.results)



# revision 58
# speedup vs baseline: 1.1962x; 1.1962x over previous
"""HEALPix downsample (scatter-mean over parent_map + 1x1 conv) on 8 Trainium2 cores.

Strategy (data-parallel over the 16 (B*C*T) slices, 2 per core):
  - Host: sort source rows by parent, pack x fp8-e3m4 (x2.5 global scale,
    max |val| ~13.5 < 15.5) with the 1/count mean weights folded into the
    rows (no padding: 49152 = 384*128). The device scatter is then a pure
    one-hot sum; fp8 halves the dominant x HBM read vs fp16 (rel err
    ~1.2e-2 vs the 2e-2 gate; e3m4's 4 mantissa bits are required - e4m3
    fails at 2.6e-2). The 1/scale is folded into the fp16 conv weight.
  - A-matrices: per 128-subtile chunk, ONE DVE is_equal op with broadcast
    APs builds all one-hot blocks [128, 128*BW] (iota tiled vs band-relative
    targets). x-independent, so off the DMA critical path.
  - Scatter: per subtile one narrow fp16 matmul psum[d, band] += X_t^T @ A_t
    into a 512-target PSUM window (one bank); bands crossing a window
    boundary issue two column-sliced matmuls. Windows zeroed by ScalarEngine.
  - 1x1 conv: fp16 matmul with W^T stationary; bias fused into the ACT
    PSUM->SBUF copy which writes fp16 output (halved output DMA).
  - The window loop is software-pipelined (tail ops lag LAG=2 windows behind
    the scatter matmuls) so no engine stalls on another's just-issued work;
    on real HW the naive per-window chain serializes and costs ~4x.
  - x chunk DMAs alternate between the SP and ACT HWDGE rings; few large
    DMAs (6 x-chunks + 6 outputs per repeat) beat many small ones.
Output is fp16 in DRAM; host upcasts to f32.
"""

import numpy as np
from contextlib import ExitStack

import concourse.bacc as bacc
import concourse.tile as tile
from concourse import mybir
from concourse.bass_utils import run_bass_kernel_spmd

# ---- problem constants (hardcoded; kernel.py must be self-contained) ----
B, C, T = 2, 2, 4
S = B * C * T                 # 16 flattened batch slices
N_SRC = 49152
N_TGT = 12288
D = 128
N_CORES = 8
S_PER_CORE = S // N_CORES     # 2

P = 128                       # partitions
NSUB = N_SRC // P             # 384 subtiles per slice (no padding)
G = 512                       # targets per psum window (one full PSUM bank)
NWIN = N_TGT // G             # 24 windows per slice
CHUNK = 128
X_PIECES = 2                  # DMA pieces per x chunk (2 -> 1MB pieces)
OUT_WINS = 4

F32 = mybir.dt.float32
F16 = mybir.dt.float16
F8 = mybir.dt.float8e3          # e3m4: 4 mantissa bits, max 15.5
SCALE = 2.5                     # folded into x on host, undone via conv weight

# experiment knobs
# GPSIMD cannot touch PSUM (HW restriction), so the three per-window PSUM
# passes (memzero, mean copy, bias+convert) are split evenly across ACT and
# DVE by window parity; Pool builds the A one-hots (SBUF-only work).
MZ_PATTERN = ("scalar", "vector")
CP_PATTERN = ("vector", "scalar")
BIAS_PATTERN = ("scalar", "vector")
# A chunks are built on DVE (Pool rejects TensorTensor) in half-chunk
# pieces, interleaved into the first windows of slice 0; while they run,
# the copy/bias tail stays off DVE entirely.
AB_ENGINE = "vector"
AB_AT = {0: [(0, 0), (0, 1)], 2: [(1, 0)], 5: [(1, 1)], 8: [(2, 0)],
         11: [(2, 1)]}
AB_WINDOWS = 16
# Ring split by direction: sync ring streams x-chunk loads back-to-back
# (issued up front per slice; all 6 chunk tiles are SBUF-resident), scalar
# ring takes consts + output stores.  Sharing a FIFO ring between loads and
# compute-gated stores head-of-line blocks the loads.
OUT_QUEUE = "gpsimd"          # "sync" | "scalar" | "gpsimd" (SWDGE)
LAG = 2                       # windows between scatter matmuls and copy/conv
BLAG = 1                      # extra windows between conv and bias/out


def _plan(parent_map):
    """Host-side metadata: packed row order, band-relative targets, matmuls."""
    pm = np.asarray(parent_map).astype(np.int64).ravel()
    assert pm.shape == (N_SRC,)
    perm = np.argsort(pm, kind="stable")
    sp = pm[perm]
    cnt = np.bincount(pm, minlength=N_TGT)
    inv = np.where(cnt > 0, 1.0 / np.maximum(cnt, 1), 0.0).astype(np.float32)

    l = sp.reshape(NSUB, P)                      # targets per (subtile, row)
    tmin = l.min(axis=1)
    tmax = l.max(axis=1)
    span = tmax - tmin + 1
    BW = int(max(32, -(-int(span.max()) // 16) * 16))   # uniform A-block width
    assert BW <= G, f"pathological parent_map: span {span.max()} > {G}"

    rel = (l - tmin[:, None]).T.astype(np.float32)      # [128, NSUB], in [0, BW)
    wrow = inv[sp]                                      # folded mean weights

    # per-subtile matmul schedule: (t, a_lo, a_hi, w, psum_off)
    sched = []
    for t in range(NSUB):
        lo = int(tmin[t])
        bw_t = min(BW, -(-int(span[t]) // 16) * 16)
        w = lo // G
        wend = (w + 1) * G
        if int(tmax[t]) < wend:
            bw_t = min(bw_t, wend - lo)               # clip phantom overhang
            sched.append((t, 0, bw_t, w, lo - w * G))
        else:                                          # band crosses window end
            cut = wend - lo
            sched.append((t, 0, cut, w, lo - w * G))
            sched.append((t, cut, bw_t, w + 1, 0))
    by_win_raw = [[] for _ in range(NWIN)]
    for m in sched:
        by_win_raw[m[3]].append(m)

    # Start-flag coverage: rewrite each window's bands into pieces such that
    # every psum column is written exactly once with start=True (the first
    # band covering it; gaps over source-less targets are covered by the
    # previous band's phantom all-zero A columns) and accumulated into by
    # the rest.  Removes the per-window PSUM memzero pass entirely.
    by_win = []
    needs_mz = []
    for w in range(NWIN):
        pieces = []
        cover = w * G
        ok = True
        prev = None                                  # (t, tmin[t]) last band
        if w > 0 and by_win_raw[w - 1]:
            tl = by_win_raw[w - 1][-1][0]
            prev = (tl, int(tmin[tl]))
        for (t, a_lo, a_hi, _, poff) in by_win_raw[w]:
            tb = int(tmin[t])
            lo_g, hi_g = tb + a_lo, tb + a_hi
            if lo_g > cover:                         # gap: zero-fill piece
                if prev is None or lo_g - prev[1] > BW:
                    ok = False
                    break
                pieces.append((prev[0], cover - prev[1], lo_g - prev[1],
                               cover - w * G, True))
            ov = max(0, min(cover, hi_g) - lo_g)
            if ov > 0:
                pieces.append((t, a_lo, a_lo + ov, lo_g - w * G, False))
            if hi_g > cover:
                st = max(cover, lo_g)
                pieces.append((t, a_lo + (st - lo_g), a_hi, st - w * G, True))
                cover = hi_g
            prev = (t, tb)
        if ok and cover < (w + 1) * G:               # tail gap
            end_g = (w + 1) * G
            if prev is None or end_g - prev[1] > BW:
                ok = False
            else:
                pieces.append((prev[0], cover - prev[1], end_g - prev[1],
                               cover - w * G, True))
        if not ok:                                   # rare fallback: memzero
            pieces = [(t, a_lo, a_hi, poff, False)
                      for (t, a_lo, a_hi, _, poff) in by_win_raw[w]]
        needs_mz.append(not ok)
        by_win.append(pieces)
    return perm, wrow, rel, (by_win, needs_mz), BW


def _build(rel_np, by_win_mz, BW, x_bufs=4, repeats=1):
    by_win, needs_mz = by_win_mz
    nc = bacc.Bacc("TRN2", target_bir_lowering=False, debug=False, enable_asserts=False)
    FREE = NSUB * P
    xp = nc.dram_tensor("xp", [S_PER_CORE, P, FREE], F8, kind="ExternalInput").ap()
    # one packed f16 const block: [rel | iota | wt | bias(f32 as 2xf16)]
    CW = NSUB + BW + P + 2
    cst = nc.dram_tensor("cst", [P, CW], F16, kind="ExternalInput").ap()
    out = nc.dram_tensor("out", [S_PER_CORE, P, N_TGT], F16, kind="ExternalOutput").ap()

    eq = mybir.AluOpType.is_equal
    n_chunks = -(-NSUB // CHUNK)

    def mz(eng_name, ap):
        if eng_name == "scalar":
            nc.scalar.memzero(ap)
        else:
            nc.vector.memset(ap, 0.0)

    out_eng = {"sync": nc.sync, "scalar": nc.scalar,
               "gpsimd": nc.gpsimd}[OUT_QUEUE]

    def cp(eng_name, out_ap, in_ap):
        if eng_name == "scalar":
            nc.scalar.copy(out=out_ap, in_=in_ap)
        else:
            eng = {"vector": nc.vector, "gpsimd": nc.gpsimd}[eng_name]
            eng.tensor_copy(out=out_ap, in_=in_ap)

    def bias_add(eng_name, out_ap, in_ap, bias_ap):
        if eng_name == "scalar":
            nc.scalar.add(out_ap, in_ap, bias_ap)
        else:
            eng = {"vector": nc.vector, "gpsimd": nc.gpsimd}[eng_name]
            eng.tensor_scalar(out=out_ap, in0=in_ap, scalar1=bias_ap,
                              scalar2=None, op0=mybir.AluOpType.add)

    with ExitStack() as ctx:
        tc = ctx.enter_context(tile.TileContext(nc))
        cpool = ctx.enter_context(tc.tile_pool(name="const", bufs=1))
        xpool = ctx.enter_context(tc.tile_pool(name="x", bufs=1))
        mpool = ctx.enter_context(tc.tile_pool(name="mean", bufs=3))
        opool = ctx.enter_context(tc.tile_pool(name="osb", bufs=1))
        pp = ctx.enter_context(tc.tile_pool(name="pp", bufs=LAG + 2, space="PSUM"))
        po = ctx.enter_context(tc.tile_pool(name="po", bufs=3, space="PSUM"))

        cst_t = cpool.tile([P, CW], F16, tag="cst")
        nc.sync.dma_start(out=cst_t[:], in_=cst[:])
        rl_t = cst_t[:, 0:NSUB]
        io_t = cst_t[:, NSUB:NSUB + BW]
        wt_t = cst_t[:, NSUB + BW:NSUB + BW + P]
        bi_t = cst_t[:, NSUB + BW + P:NSUB + BW + P + 2].bitcast(F32)

        # A is slice-invariant (parent_map only): one resident SBUF tile per
        # chunk, filled once (half-chunk builds scheduled via AB_AT below)
        # and reused for all slices/repeats.
        a_tiles = [
            cpool.tile([P, (min(FREE, (c + 1) * CHUNK * P) // P - c * CHUNK) * BW],
                       F8, tag=f"at{c}", name=f"at{c}")
            for c in range(n_chunks)
        ]

        ab_eng = {"vector": nc.vector, "gpsimd": nc.gpsimd}[AB_ENGINE]

        def build_a(c, half):
            nsub_c = a_tiles[c].shape[1] // BW
            h0 = half * (nsub_c // 2)
            h1 = nsub_c if half else nsub_c // 2
            nh = h1 - h0
            ab_eng.tensor_tensor(
                out=a_tiles[c][:, h0 * BW:h1 * BW]
                    .rearrange("p (t c) -> p t c", t=nh),
                in0=io_t[:].unsqueeze(1).broadcast_to([P, nh, BW]),
                in1=rl_t[:, c * CHUNK + h0:c * CHUNK + h1]
                    .unsqueeze(2).broadcast_to([P, nh, BW]),
                op=eq)

        for rep in range(repeats):
            # issue the whole repeat's x stream (both slices) up front on the
            # sync ring so loads never queue behind compute-gated output
            # stores; every piece has its own resident buffer.  The first
            # chunk arrives as two 1MB halves (fast pipeline fill), the rest
            # as full 2MB chunks (better HBM efficiency).
            x_tiles = {}
            xspan = CHUNK // X_PIECES
            for s in range(S_PER_CORE):
                for c in range(n_chunks):
                    parts = []
                    for h in range(X_PIECES):
                        f0 = (c * CHUNK + h * xspan) * P
                        f1 = min(FREE, f0 + xspan * P)
                        xt = xpool.tile([P, f1 - f0], F8, tag=f"xc{s}_{c}_{h}",
                                        name=f"xc{s}_{c}_{h}")
                        nc.sync.dma_start(out=xt[:], in_=xp[s, :, f0:f1])
                        parts.append(xt)
                    x_tiles[(s, c)] = parts

            def x_slice(s, c, loc):
                h, lh = divmod(loc, xspan)
                return x_tiles[(s, c)][h][:, lh * P:(lh + 1) * P]

            # software-pipelined, two-stage tail: iter w issues the scatter
            # matmuls for window w, the copy+conv for window w-LAG, and the
            # bias+out-DMA for window w-LAG-BLAG.  Splitting copy and bias
            # into different lag stages keeps ACT from stalling on the PE
            # conv it just fed (the conv sits behind window w's scatter
            # matmuls in PE program order).
            for s in range(S_PER_CORE):
                ps_tiles = {}
                pso_tiles = {}
                # full-slice output staging: biases land here unconditionally
                # (never blocked on out-DMA completion); the sync ring drains
                # OUT_WINS-window pieces behind the x stream at its own pace.
                out_t = opool.tile([P, N_TGT], F16, tag=f"os{s}", name=f"os{s}")
                for w in range(NWIN + LAG + BLAG):
                    first = rep == 0 and s == 0
                    if first:
                        for c_h in AB_AT.get(w, ()):
                            build_a(*c_h)
                    if w < NWIN:
                        mlist = by_win[w]
                        ps = pp.tile([P, G], F32, tag="pp")
                        if needs_mz[w]:
                            mz(MZ_PATTERN[w % len(MZ_PATTERN)], ps[:])
                        ps_tiles[w] = ps
                        nlast = len(mlist) - 1
                        for i, (t, a_lo, a_hi, poff, st) in enumerate(mlist):
                            c, loc = divmod(t, CHUNK)
                            nc.tensor.matmul(
                                out=ps[:, poff:poff + (a_hi - a_lo)],
                                lhsT=x_slice(s, c, loc),
                                rhs=a_tiles[c][:, loc * BW + a_lo:loc * BW + a_hi],
                                start=st, stop=(i == nlast),
                                skip_group_check=True)
                    wt_ = w - LAG
                    if 0 <= wt_ < NWIN:
                        ps_ = ps_tiles.pop(wt_)
                        mean_t = mpool.tile([P, G], F16, tag="mt")
                        cpe = ("scalar" if first and wt_ < AB_WINDOWS
                               else CP_PATTERN[wt_ % len(CP_PATTERN)])
                        cp(cpe, mean_t[:], ps_[:])
                        pso = po.tile([P, G], F32, tag="po")
                        nc.tensor.matmul(out=pso[:], lhsT=wt_t, rhs=mean_t[:],
                                         start=True, stop=True)
                        pso_tiles[wt_] = pso
                    wb = w - LAG - BLAG
                    if wb < 0:
                        continue
                    be = ("scalar" if first and wb < AB_WINDOWS
                          else BIAS_PATTERN[wb % len(BIAS_PATTERN)])
                    bias_add(be, out_t[:, wb * G:(wb + 1) * G],
                             pso_tiles.pop(wb)[:], bi_t[:, 0:1])
                    if wb % OUT_WINS == OUT_WINS - 1:
                        w0 = wb - (OUT_WINS - 1)
                        out_eng.dma_start(out=out[s, :, w0 * G:(wb + 1) * G],
                                          in_=out_t[:, w0 * G:(wb + 1) * G])
    nc.compile()
    return nc


_CACHE = {}


def _prepare(parent_map):
    key = np.asarray(parent_map).astype(np.int64).tobytes()
    entry = _CACHE.get(key)
    if entry is None:
        perm, wrow, rel, by_win, BW = _plan(parent_map)
        nc = _build(rel, by_win, BW)
        entry = (nc, perm, wrow, rel, BW)
        _CACHE[key] = entry
        _CACHE[(key, "plan")] = (perm, wrow, rel, by_win, BW)
    return entry


def build_repeated(parent_map, repeats):
    """Benchmark variant: same program with the whole body repeated."""
    _prepare(parent_map)
    key = np.asarray(parent_map).astype(np.int64).tobytes()
    perm, wrow, rel, by_win, BW = _CACHE[(key, "plan")]
    return _build(rel, by_win, BW, repeats=repeats)


def make_in_maps(x, parent_map, W, b):
    """Pack full inputs into the 8 per-core input maps."""
    import ml_dtypes
    f8 = ml_dtypes.float8_e3m4
    nc, perm, wrow, rel, BW = _prepare(parent_map)
    x2 = np.asarray(x, np.float32).reshape(S, N_SRC, D)
    FREE = NSUB * P
    wcol = (wrow.astype(np.float32) * SCALE)[:, None]
    xp_all = np.empty((S, P, FREE), f8)
    for s in range(S):
        g = (x2[s][perm] * wcol).astype(f8).reshape(NSUB, P, D)
        xp_all[s] = g.transpose(1, 0, 2).reshape(P, FREE)
    wt = (np.asarray(W, np.float32).T / SCALE).astype(np.float16)
    bias = np.asarray(b, np.float32).reshape(P, 1)
    iota = np.broadcast_to(np.arange(BW, dtype=np.float16), (P, BW))
    rel_c = rel.astype(np.float16)
    cst = np.ascontiguousarray(np.concatenate(
        [rel_c, iota, wt, bias.view(np.float16)], axis=1))
    in_maps = []
    for c in range(N_CORES):
        in_maps.append({
            "xp": np.ascontiguousarray(xp_all[c * S_PER_CORE:(c + 1) * S_PER_CORE]),
            "cst": cst,
        })
    return nc, in_maps


def assemble_output(results):
    """results: per-core list of {"out": [S_PER_CORE, P, N_TGT]} -> full output."""
    outs = np.stack([np.asarray(results[c]["out"]) for c in range(N_CORES)])
    out_full = outs.astype(np.float32).reshape(S, P, N_TGT).transpose(0, 2, 1)
    return np.ascontiguousarray(out_full).reshape(B, C, T, N_TGT, D)


def kernel(x, parent_map, W, b):
    nc, in_maps = make_in_maps(x, parent_map, W, b)
    res = run_bass_kernel_spmd(nc, in_maps, list(range(N_CORES)))
    return assemble_output(res.results)

